# revision 8
# baseline (speedup 1.0000x reference)
"""AtomUpdateBlock Trainium2 kernel (8 NeuronCores, SPMD, no collectives).

Strategy:
  - Host: sort edges by destination atom id_j, shard by atom range (6250
    atoms/core).  Within a core, atoms are grouped into 49 blocks of 128;
    each block's edge list is padded to T*128 edges (T = global max tiles
    per block) and packed in tile-major layout for perfect DMA patterns.
  - Device, stage 1 (edges): per 128-edge tile,
        mlp = rbf_tile @ W_rbf           (PE, K=16)
        x   = m_tile * mlp               (ACT copy + DVE mult, bf16)
        oh[e,a] = (id[e] == a)           (GPSIMD tensor_scalar vs iota)
        x2_block += oh.T @ x             (PE, accumulated in PSUM)
    Segment-sum needs no collective because the atom range owns all its
    edges (host sorted them).
  - Device, stage 2 (atoms): transpose x2 blocks to feature-major and run
    the 5 dense layers (W1 + 2 residual blocks) weight-stationary across
    blocks, silu on ACT, residual merges on DVE.
  - Host: concat per-core outputs.
"""

import sys

if "/opt/trn_rl_repo" not in sys.path:
    sys.path.insert(0, "/opt/trn_rl_repo")

import numpy as np
import ml_dtypes

N_CORES = 8
N_ATOMS = 50000
N_EDGES = 1600000
D = 256          # edge/atom embedding
R = 16           # rbf dim
P = 128
ATOMS_PER_CORE = N_ATOMS // N_CORES      # 6250
NBLK = (ATOMS_PER_CORE + P - 1) // P     # 49 blocks of 128 atoms
ATOMS_PAD = NBLK * P                     # 6272
INV_SQRT2 = 0.7071067811865476
PAD_ID = 200.0                           # local id that matches no iota value

BF16 = ml_dtypes.bfloat16

_COMPILED = {}   # T -> nc


def _pack_inputs(m, rbf, id_j):
    """Sort edges by atom, shard by atom range, pad per 128-atom block."""
    id_j = np.ascontiguousarray(np.asarray(id_j).astype(np.int64).ravel())
    order = np.argsort(id_j)
    ids_sorted = id_j[order]

    # block atom boundaries for every (core, block)
    blk_bases = []           # absolute atom id at start of each block
    for c in range(N_CORES):
        for b in range(NBLK):
            blk_bases.append(c * ATOMS_PER_CORE + min(b * P, ATOMS_PER_CORE))
    blk_bases.append(N_ATOMS)
    blk_bases = np.asarray(blk_bases, dtype=np.int64)
    bounds = np.searchsorted(ids_sorted, blk_bases)
    cnts = np.diff(bounds).reshape(N_CORES, NBLK)    # edges per (core, block)

    T = int(np.ceil(cnts.max() / P))
    E_blk = T * P
    NT = NBLK * T

    m_bf = np.asarray(m).astype(BF16)
    rbf_bf = np.asarray(rbf).astype(BF16)

    in_parts = []
    for c in range(N_CORES):
        s, e = bounds[c * NBLK], bounds[(c + 1) * NBLK]
        n_c = e - s
        ord_c = order[s:e]
        cnt_c = cnts[c]
        blk_of = np.repeat(np.arange(NBLK), cnt_c)
        # destination slot inside the padded layout
        starts_rel = np.concatenate([[0], np.cumsum(cnt_c)[:-1]])
        j = np.arange(n_c)
        dest = blk_of * E_blk + (j - starts_rel[blk_of])

        m_core = np.zeros((NBLK * E_blk, D), dtype=BF16)
        m_core[dest] = m_bf[ord_c]
        # tile-major: [NBLK, T, P, D] -> [NBLK, P, T, D] -> [NBLK*P, T*D]
        m_core = np.ascontiguousarray(
            m_core.reshape(NBLK, T, P, D).transpose(0, 2, 1, 3)
        ).reshape(NBLK * P, T * D)

        rbf_core = np.zeros((NBLK * E_blk, R), dtype=BF16)
        rbf_core[dest] = rbf_bf[ord_c]
        # [NBLK, T, P, R] -> [NBLK, R, T, P] -> [NBLK*R, T*P]
        rbf_core = np.ascontiguousarray(
            rbf_core.reshape(NBLK, T, P, R).transpose(0, 3, 1, 2)
        ).reshape(NBLK * R, T * P)

        ids_core = np.full(NBLK * E_blk, PAD_ID, dtype=np.float32)
        ids_core[dest] = (ids_sorted[s:e] - (c * ATOMS_PER_CORE + P * blk_of)).astype(
            np.float32
        )
        # [NBLK, T, P] -> [P, NBLK, T] -> [P, NT]
        ids_core = np.ascontiguousarray(
            ids_core.reshape(NBLK, T, P).transpose(2, 0, 1)
        ).reshape(P, NT)

        in_parts.append({"m": m_core, "rbf": rbf_core, "ids": ids_core})
    return in_parts, T


def _pack_weights(W_rbf, W1, W_res):
    w_rbf = np.asarray(W_rbf).astype(BF16)                      # [16, 256]
    layers = [np.asarray(W1)] + [
        np.asarray(W_res[i, l]) for i in range(W_res.shape[0]) for l in range(2)
    ]
    # chunks[l, dc, uc] = W_l[dc*128:, uc*128:]  -> lhsT [K=128 d, M=128 u]
    w2 = np.zeros((5, 2, 2, P, P), dtype=BF16)
    for l, W in enumerate(layers):
        for dc in range(2):
            for uc in range(2):
                w2[l, dc, uc] = W[dc * P : (dc + 1) * P, uc * P : (uc + 1) * P].astype(
                    BF16
                )
    w2 = w2.reshape(20 * P, P)

    iota = np.tile(np.arange(P, dtype=np.float32), (P, 1)).astype(BF16)  # [P, P]
    ident = np.eye(P, dtype=np.float32).astype(BF16)
    return {"w_rbf": w_rbf, "w2": w2, "iota": iota, "ident": ident}


def _build(T):
    from concourse import mybir, bacc
    import concourse.tile as tile

    f32 = mybir.dt.float32
    bf16 = mybir.dt.bfloat16
    AF = mybir.ActivationFunctionType
    ALU = mybir.AluOpType
    NT = NBLK * T

    nc = bacc.Bacc("TRN2", target_bir_lowering=False, debug=False,
                   num_devices=N_CORES)

    m_d = nc.dram_tensor("m", [NBLK * P, T * D], bf16, kind="ExternalInput").ap()
    rbf_d = nc.dram_tensor("rbf", [NBLK * R, T * P], bf16, kind="ExternalInput").ap()
    ids_d = nc.dram_tensor("ids", [P, NT], f32, kind="ExternalInput").ap()
    wrbf_d = nc.dram_tensor("w_rbf", [R, D], bf16, kind="ExternalInput").ap()
    w2_d = nc.dram_tensor("w2", [20 * P, P], bf16, kind="ExternalInput").ap()
    iota_d = nc.dram_tensor("iota", [P, P], bf16, kind="ExternalInput").ap()
    ident_d = nc.dram_tensor("ident", [P, P], bf16, kind="ExternalInput").ap()
    out_d = nc.dram_tensor("out", [2 * P, ATOMS_PAD], f32, kind="ExternalOutput").ap()

    with tile.TileContext(nc) as tc, \
         tc.tile_pool(name="const", bufs=1) as cpool, \
         tc.tile_pool(name="mblk", bufs=2) as mpool, \
         tc.tile_pool(name="rbfblk", bufs=2) as rpool, \
         tc.tile_pool(name="small", bufs=4) as spool, \
         tc.tile_pool(name="ohp", bufs=4) as ohpool, \
         tc.tile_pool(name="big", bufs=1) as bigpool, \
         tc.tile_pool(name="outp", bufs=4) as opool:
        # ---- constants ----
        w_rbf_sb = cpool.tile([R, D], bf16, tag="w_rbf")
        nc.sync.dma_start(w_rbf_sb[:], wrbf_d[:])
        w2_sb = cpool.tile([P, 20 * P], bf16, tag="w2")
        for i in range(20):
            nc.sync.dma_start(
                w2_sb[:, i * P : (i + 1) * P], w2_d[i * P : (i + 1) * P, :]
            )
        iota_sb = cpool.tile([P, P], bf16, tag="iota")
        nc.sync.dma_start(iota_sb[:], iota_d[:])
        ident_sb = cpool.tile([P, P], bf16, tag="ident")
        nc.sync.dma_start(ident_sb[:], ident_d[:])
        ids_sb = cpool.tile([P, NT], f32, tag="ids")
        nc.sync.dma_start(ids_sb[:], ids_d[:])

        def w2c(l, dc, uc):
            i = (l * 2 + dc) * 2 + uc
            return w2_sb[:, i * P : (i + 1) * P]

        # persistent feature-major activations [2 chunks][P, ATOMS_PAD]
        A = [bigpool.tile([P, ATOMS_PAD], bf16, tag=f"A{c}", name=f"A{c}")
             for c in range(2)]
        B = [bigpool.tile([P, ATOMS_PAD], bf16, tag=f"B{c}", name=f"B{c}")
             for c in range(2)]
        C = [bigpool.tile([P, ATOMS_PAD], bf16, tag=f"C{c}", name=f"C{c}")
             for c in range(2)]

        # ---------------- stage 1: edges -> x2^T (into A) ----------------
        with tc.tile_pool(name="ps_mlp", bufs=2, space="PSUM") as ps_mlp, \
             tc.tile_pool(name="ps_x2", bufs=2, space="PSUM") as ps_x2, \
             tc.tile_pool(name="ps_t", bufs=2, space="PSUM") as ps_t:
            for b in range(NBLK):
                m_blk = mpool.tile([P, T * D], bf16, tag="m")
                nc.sync.dma_start(m_blk[:], m_d[b * P : (b + 1) * P, :])
                rbf_blk = rpool.tile([R, T * P], bf16, tag="rbf")
                nc.sync.dma_start(rbf_blk[:], rbf_d[b * R : (b + 1) * R, :])

                x2_ps = ps_x2.tile([P, D], f32, tag="x2")
                for t in range(T):
                    tb = b * T + t
                    mlp_ps = ps_mlp.tile([P, D], f32, tag="mlp")
                    nc.tensor.matmul(
                        mlp_ps[:],
                        lhsT=rbf_blk[:, t * P : (t + 1) * P],
                        rhs=w_rbf_sb[:],
                        start=True,
                        stop=True,
                    )
                    mlp_sb = spool.tile([P, D], bf16, tag="mlp_sb")
                    nc.scalar.copy(mlp_sb[:], mlp_ps[:])
                    x_sb = spool.tile([P, D], bf16, tag="x_sb")
                    nc.vector.tensor_mul(
                        x_sb[:], m_blk[:, t * D : (t + 1) * D], mlp_sb[:]
                    )
                    oh = ohpool.tile([P, P], bf16, tag="oh")
                    nc.gpsimd.tensor_scalar(
                        oh[:], iota_sb[:], ids_sb[:, tb : tb + 1], None,
                        op0=ALU.is_equal,
                    )
                    nc.tensor.matmul(
                        x2_ps[:], lhsT=oh[:], rhs=x_sb[:],
                        start=(t == 0), stop=(t == T - 1),
                    )
                x2_sb = spool.tile([P, D], bf16, tag="x2_sb")
                nc.scalar.copy(x2_sb[:], x2_ps[:])
                for ch in range(2):
                    tp = ps_t.tile([P, P], bf16, tag="tp")
                    nc.tensor.transpose(
                        tp[:], x2_sb[:, ch * P : (ch + 1) * P], ident_sb[:]
                    )
                    nc.vector.tensor_copy(A[ch][:, b * P : (b + 1) * P], tp[:])

        # ---------------- stage 2: atom MLP (feature-major) --------------
        with tc.tile_pool(name="ps_s2", bufs=4, space="PSUM") as ps_s2:
            G = 4

            def dense(l, IN, OUT):
                for uc in range(2):
                    for g0 in range(0, NBLK, G):
                        g1 = min(g0 + G, NBLK)
                        pss = [
                            ps_s2.tile([P, P], f32, tag="s2", name=f"s2_{l}_{uc}_{b}")
                            for b in range(g0, g1)
                        ]
                        for dc in range(2):
                            for i, b in enumerate(range(g0, g1)):
                                nc.tensor.matmul(
                                    pss[i][:],
                                    lhsT=w2c(l, dc, uc),
                                    rhs=IN[dc][:, b * P : (b + 1) * P],
                                    start=(dc == 0),
                                    stop=(dc == 1),
                                )
                        for i, b in enumerate(range(g0, g1)):
                            nc.scalar.activation(
                                OUT[uc][:, b * P : (b + 1) * P], pss[i][:], AF.Silu
                            )

            dense(0, A, B)     # h0 = silu(x2 @ W1)
            dense(1, B, C)     # u1 = silu(h0 @ Wa0)
            dense(2, C, A)     # v1 = silu(u1 @ Wb0)
            # merge1: C = (B + A) * inv_sqrt2
            for ch in range(2):
                nc.vector.tensor_add(C[ch][:], B[ch][:], A[ch][:])
                nc.vector.tensor_scalar_mul(C[ch][:], C[ch][:], INV_SQRT2)
            dense(3, C, B)     # u2 = silu(h1 @ Wa1)
            dense(4, B, A)     # v2 = silu(u2 @ Wb1)
            # final: out = (C + A) * inv_sqrt2   (f32), DMA per block
            for ch in range(2):
                for b in range(NBLK):
                    o = opool.tile([P, P], f32, tag="o")
                    nc.vector.tensor_add(
                        o[:], C[ch][:, b * P : (b + 1) * P],
                        A[ch][:, b * P : (b + 1) * P],
                    )
                    nc.vector.tensor_scalar_mul(o[:], o[:], INV_SQRT2)
                    nc.sync.dma_start(
                        out_d[ch * P : (ch + 1) * P, b * P : (b + 1) * P], o[:]
                    )

    nc.compile()
    return nc


def _run(inputs, trace=False):
    from concourse.bass_utils import run_bass_kernel_spmd

    m, rbf, id_j = inputs["m"], inputs["rbf"], inputs["id_j"]
    in_parts, T = _pack_inputs(m, rbf, id_j)
    consts = _pack_weights(inputs["W_rbf"], inputs["W1"], inputs["W_res"])

    if T not in _COMPILED:
        _COMPILED[T] = _build(T)
    nc = _COMPILED[T]

    in_maps = [{**part, **consts} for part in in_parts]
    res = run_bass_kernel_spmd(
        nc, in_maps, core_ids=list(range(N_CORES)), trace=trace
    )
    outs = []
    for c in range(N_CORES):
        o = res.results[c]["out"]                      # [256, ATOMS_PAD]
        outs.append(o[:, :ATOMS_PER_CORE].T)           # [6250, 256]
    full = np.concatenate(outs, axis=0).astype(np.float32)
    return full, res.exec_time_ns


def kernel(**inputs):
    out, _ = _run(inputs, trace=False)
    return out


# revision 9
# speedup vs baseline: 2.4177x; 2.4177x over previous
"""AtomUpdateBlock Trainium2 kernel (8 NeuronCores, SPMD, no collectives).

Strategy:
  - Host: sort edges by destination atom id_j, shard by atom range (6250
    atoms/core).  Within a core, atoms are grouped into 49 blocks of 128;
    each block's edge list is padded to T*128 edges (T = global max tiles
    per block) and packed in tile-major layout for perfect DMA patterns.
  - Device, stage 1 (edges): per 128-edge tile,
        mlp = rbf_tile @ W_rbf           (PE, K=16)
        x   = m_tile * mlp               (ACT copy + DVE mult, bf16)
        oh[e,a] = (id[e] == a)           (GPSIMD tensor_scalar vs iota)
        x2_block += oh.T @ x             (PE, accumulated in PSUM)
    Segment-sum needs no collective because the atom range owns all its
    edges (host sorted them).
  - Device, stage 2 (atoms): transpose x2 blocks to feature-major and run
    the 5 dense layers (W1 + 2 residual blocks) weight-stationary across
    blocks, silu on ACT, residual merges on DVE.
  - Host: concat per-core outputs.
"""

import sys

if "/opt/trn_rl_repo" not in sys.path:
    sys.path.insert(0, "/opt/trn_rl_repo")

import numpy as np
import ml_dtypes

N_CORES = 8
N_ATOMS = 50000
N_EDGES = 1600000
D = 256          # edge/atom embedding
R = 16           # rbf dim
P = 128
ATOMS_PER_CORE = N_ATOMS // N_CORES      # 6250
NBLK = (ATOMS_PER_CORE + P - 1) // P     # 49 blocks of 128 atoms
ATOMS_PAD = NBLK * P                     # 6272
INV_SQRT2 = 0.7071067811865476
PAD_ID = 200.0                           # local id that matches no iota value

BF16 = ml_dtypes.bfloat16

_COMPILED = {}   # T -> nc


def _pack_inputs(m, rbf, id_j):
    """Sort edges by atom, shard by atom range, pad per 128-atom block."""
    id_j = np.ascontiguousarray(np.asarray(id_j).astype(np.int64).ravel())
    order = np.argsort(id_j)
    ids_sorted = id_j[order]

    # block atom boundaries for every (core, block)
    blk_bases = []           # absolute atom id at start of each block
    for c in range(N_CORES):
        for b in range(NBLK):
            blk_bases.append(c * ATOMS_PER_CORE + min(b * P, ATOMS_PER_CORE))
    blk_bases.append(N_ATOMS)
    blk_bases = np.asarray(blk_bases, dtype=np.int64)
    bounds = np.searchsorted(ids_sorted, blk_bases)
    cnts = np.diff(bounds).reshape(N_CORES, NBLK)    # edges per (core, block)

    T = int(np.ceil(cnts.max() / P))
    E_blk = T * P
    NT = NBLK * T

    m_bf = np.asarray(m).astype(BF16)
    rbf_bf = np.asarray(rbf).astype(BF16)

    in_parts = []
    for c in range(N_CORES):
        s, e = bounds[c * NBLK], bounds[(c + 1) * NBLK]
        n_c = e - s
        ord_c = order[s:e]
        cnt_c = cnts[c]
        blk_of = np.repeat(np.arange(NBLK), cnt_c)
        # destination slot inside the padded layout
        starts_rel = np.concatenate([[0], np.cumsum(cnt_c)[:-1]])
        j = np.arange(n_c)
        dest = blk_of * E_blk + (j - starts_rel[blk_of])

        m_core = np.zeros((NBLK * E_blk, D), dtype=BF16)
        m_core[dest] = m_bf[ord_c]
        # tile-major: [NBLK, T, P, D] -> [NBLK, P, T, D] -> [NBLK*P, T*D]
        m_core = np.ascontiguousarray(
            m_core.reshape(NBLK, T, P, D).transpose(0, 2, 1, 3)
        ).reshape(NBLK * P, T * D)

        rbf_core = np.zeros((NBLK * E_blk, R), dtype=BF16)
        rbf_core[dest] = rbf_bf[ord_c]
        # [NBLK, T, P, R] -> [NBLK, R, T, P] -> [NBLK*R, T*P]
        rbf_core = np.ascontiguousarray(
            rbf_core.reshape(NBLK, T, P, R).transpose(0, 3, 1, 2)
        ).reshape(NBLK * R, T * P)

        ids_core = np.full(NBLK * E_blk, PAD_ID, dtype=np.float32)
        ids_core[dest] = (ids_sorted[s:e] - (c * ATOMS_PER_CORE + P * blk_of)).astype(
            np.float32
        )
        # [NBLK, T, P] -> [P, NBLK, T] -> [P, NT]
        ids_core = np.ascontiguousarray(
            ids_core.reshape(NBLK, T, P).transpose(2, 0, 1)
        ).reshape(P, NT)

        in_parts.append({"m": m_core, "rbf": rbf_core, "ids": ids_core})
    return in_parts, T


def _pack_weights(W_rbf, W1, W_res):
    w_rbf = np.asarray(W_rbf).astype(BF16)                      # [16, 256]
    layers = [np.asarray(W1)] + [
        np.asarray(W_res[i, l]) for i in range(W_res.shape[0]) for l in range(2)
    ]
    # chunks[l, dc, uc] = W_l[dc*128:, uc*128:]  -> lhsT [K=128 d, M=128 u]
    w2 = np.zeros((5, 2, 2, P, P), dtype=BF16)
    for l, W in enumerate(layers):
        for dc in range(2):
            for uc in range(2):
                w2[l, dc, uc] = W[dc * P : (dc + 1) * P, uc * P : (uc + 1) * P].astype(
                    BF16
                )
    w2 = w2.reshape(20 * P, P)

    iota = np.tile(np.arange(P, dtype=np.float32), (P, 1)).astype(BF16)  # [P, P]
    ident = np.eye(P, dtype=np.float32).astype(BF16)
    return {"w_rbf": w_rbf, "w2": w2, "iota": iota, "ident": ident}


def _build(T):
    from concourse import mybir, bacc
    import concourse.tile as tile

    f32 = mybir.dt.float32
    bf16 = mybir.dt.bfloat16
    AF = mybir.ActivationFunctionType
    ALU = mybir.AluOpType
    NT = NBLK * T

    nc = bacc.Bacc("TRN2", target_bir_lowering=False, debug=False,
                   num_devices=N_CORES)

    m_d = nc.dram_tensor("m", [NBLK * P, T * D], bf16, kind="ExternalInput").ap()
    rbf_d = nc.dram_tensor("rbf", [NBLK * R, T * P], bf16, kind="ExternalInput").ap()
    ids_d = nc.dram_tensor("ids", [P, NT], f32, kind="ExternalInput").ap()
    wrbf_d = nc.dram_tensor("w_rbf", [R, D], bf16, kind="ExternalInput").ap()
    w2_d = nc.dram_tensor("w2", [20 * P, P], bf16, kind="ExternalInput").ap()
    iota_d = nc.dram_tensor("iota", [P, P], bf16, kind="ExternalInput").ap()
    ident_d = nc.dram_tensor("ident", [P, P], bf16, kind="ExternalInput").ap()
    out_d = nc.dram_tensor("out", [2 * P, ATOMS_PAD], f32, kind="ExternalOutput").ap()

    with tile.TileContext(nc) as tc, \
         tc.tile_pool(name="const", bufs=1) as cpool, \
         tc.tile_pool(name="mblk", bufs=2) as mpool, \
         tc.tile_pool(name="rbfblk", bufs=2) as rpool, \
         tc.tile_pool(name="small", bufs=4) as spool, \
         tc.tile_pool(name="ohp", bufs=4) as ohpool, \
         tc.tile_pool(name="big", bufs=1) as bigpool, \
         tc.tile_pool(name="outp", bufs=4) as opool:
        # ---- constants ----
        w_rbf_sb = cpool.tile([R, D], bf16, tag="w_rbf")
        nc.sync.dma_start(w_rbf_sb[:], wrbf_d[:])
        w2_sb = cpool.tile([P, 20 * P], bf16, tag="w2")
        for i in range(20):
            nc.sync.dma_start(
                w2_sb[:, i * P : (i + 1) * P], w2_d[i * P : (i + 1) * P, :]
            )
        iota_sb = cpool.tile([P, P], bf16, tag="iota")
        nc.sync.dma_start(iota_sb[:], iota_d[:])
        ident_sb = cpool.tile([P, P], bf16, tag="ident")
        nc.sync.dma_start(ident_sb[:], ident_d[:])
        ids_sb = cpool.tile([P, NT], f32, tag="ids")
        nc.sync.dma_start(ids_sb[:], ids_d[:])

        def w2c(l, dc, uc):
            i = (l * 2 + dc) * 2 + uc
            return w2_sb[:, i * P : (i + 1) * P]

        # persistent feature-major activations [2 chunks][P, ATOMS_PAD]
        A = [bigpool.tile([P, ATOMS_PAD], bf16, tag=f"A{c}", name=f"A{c}")
             for c in range(2)]
        B = [bigpool.tile([P, ATOMS_PAD], bf16, tag=f"B{c}", name=f"B{c}")
             for c in range(2)]
        C = [bigpool.tile([P, ATOMS_PAD], bf16, tag=f"C{c}", name=f"C{c}")
             for c in range(2)]

        # ---------------- stage 1: edges -> x2^T (into A) ----------------
        with tc.tile_pool(name="ps_mlp", bufs=2, space="PSUM") as ps_mlp, \
             tc.tile_pool(name="ps_x2", bufs=2, space="PSUM") as ps_x2, \
             tc.tile_pool(name="ps_t", bufs=2, space="PSUM") as ps_t:
            for b in range(NBLK):
                m_blk = mpool.tile([P, T * D], bf16, tag="m")
                nc.sync.dma_start(m_blk[:], m_d[b * P : (b + 1) * P, :])
                rbf_blk = rpool.tile([R, T * P], bf16, tag="rbf")
                nc.sync.dma_start(rbf_blk[:], rbf_d[b * R : (b + 1) * R, :])

                x2_ps = ps_x2.tile([P, D], f32, tag="x2")
                for t in range(T):
                    tb = b * T + t
                    mlp_ps = ps_mlp.tile([P, D], f32, tag="mlp")
                    nc.tensor.matmul(
                        mlp_ps[:],
                        lhsT=rbf_blk[:, t * P : (t + 1) * P],
                        rhs=w_rbf_sb[:],
                        start=True,
                        stop=True,
                    )
                    mlp_sb = spool.tile([P, D], bf16, tag="mlp_sb")
                    nc.scalar.copy(mlp_sb[:], mlp_ps[:])
                    x_sb = spool.tile([P, D], bf16, tag="x_sb")
                    nc.vector.tensor_mul(
                        x_sb[:], m_blk[:, t * D : (t + 1) * D], mlp_sb[:]
                    )
                    oh = ohpool.tile([P, P], bf16, tag="oh")
                    nc.vector.tensor_scalar(
                        oh[:], iota_sb[:], ids_sb[:, tb : tb + 1], None,
                        op0=ALU.is_equal,
                    )
                    nc.tensor.matmul(
                        x2_ps[:], lhsT=oh[:], rhs=x_sb[:],
                        start=(t == 0), stop=(t == T - 1),
                    )
                x2_sb = spool.tile([P, D], bf16, tag="x2_sb")
                nc.scalar.copy(x2_sb[:], x2_ps[:])
                for ch in range(2):
                    tp = ps_t.tile([P, P], bf16, tag="tp")
                    nc.tensor.transpose(
                        tp[:], x2_sb[:, ch * P : (ch + 1) * P], ident_sb[:]
                    )
                    nc.vector.tensor_copy(A[ch][:, b * P : (b + 1) * P], tp[:])

        # ---------------- stage 2: atom MLP (feature-major) --------------
        with tc.tile_pool(name="ps_s2", bufs=4, space="PSUM") as ps_s2:
            G = 4

            def dense(l, IN, OUT):
                for uc in range(2):
                    for g0 in range(0, NBLK, G):
                        g1 = min(g0 + G, NBLK)
                        pss = [
                            ps_s2.tile([P, P], f32, tag="s2", name=f"s2_{l}_{uc}_{b}")
                            for b in range(g0, g1)
                        ]
                        for dc in range(2):
                            for i, b in enumerate(range(g0, g1)):
                                nc.tensor.matmul(
                                    pss[i][:],
                                    lhsT=w2c(l, dc, uc),
                                    rhs=IN[dc][:, b * P : (b + 1) * P],
                                    start=(dc == 0),
                                    stop=(dc == 1),
                                )
                        for i, b in enumerate(range(g0, g1)):
                            nc.scalar.activation(
                                OUT[uc][:, b * P : (b + 1) * P], pss[i][:], AF.Silu
                            )

            dense(0, A, B)     # h0 = silu(x2 @ W1)
            dense(1, B, C)     # u1 = silu(h0 @ Wa0)
            dense(2, C, A)     # v1 = silu(u1 @ Wb0)
            # merge1: C = (B + A) * inv_sqrt2
            for ch in range(2):
                nc.vector.tensor_add(C[ch][:], B[ch][:], A[ch][:])
                nc.vector.tensor_scalar_mul(C[ch][:], C[ch][:], INV_SQRT2)
            dense(3, C, B)     # u2 = silu(h1 @ Wa1)
            dense(4, B, A)     # v2 = silu(u2 @ Wb1)
            # final: out = (C + A) * inv_sqrt2   (f32), DMA per block
            for ch in range(2):
                for b in range(NBLK):
                    o = opool.tile([P, P], f32, tag="o")
                    nc.vector.tensor_add(
                        o[:], C[ch][:, b * P : (b + 1) * P],
                        A[ch][:, b * P : (b + 1) * P],
                    )
                    nc.vector.tensor_scalar_mul(o[:], o[:], INV_SQRT2)
                    nc.sync.dma_start(
                        out_d[ch * P : (ch + 1) * P, b * P : (b + 1) * P], o[:]
                    )

    nc.compile()
    return nc


def _run(inputs, trace=False):
    from concourse.bass_utils import run_bass_kernel_spmd

    m, rbf, id_j = inputs["m"], inputs["rbf"], inputs["id_j"]
    in_parts, T = _pack_inputs(m, rbf, id_j)
    consts = _pack_weights(inputs["W_rbf"], inputs["W1"], inputs["W_res"])

    if T not in _COMPILED:
        _COMPILED[T] = _build(T)
    nc = _COMPILED[T]

    in_maps = [{**part, **consts} for part in in_parts]
    res = run_bass_kernel_spmd(
        nc, in_maps, core_ids=list(range(N_CORES)), trace=trace
    )
    outs = []
    for c in range(N_CORES):
        o = res.results[c]["out"]                      # [256, ATOMS_PAD]
        outs.append(o[:, :ATOMS_PER_CORE].T)           # [6250, 256]
    full = np.concatenate(outs, axis=0).astype(np.float32)
    return full, res.exec_time_ns


def kernel(**inputs):
    out, _ = _run(inputs, trace=False)
    return out


# revision 11
# speedup vs baseline: 3.0219x; 1.2499x over previous
"""AtomUpdateBlock Trainium2 kernel (8 NeuronCores, SPMD, no collectives).

Strategy:
  - Host: sort edges by destination atom id_j, shard by atom range (6250
    atoms/core).  Within a core, atoms are grouped into 49 blocks of 128;
    each block's edge list is padded to T*128 edges (T = global max tiles
    per block) and packed in tile-major layout for perfect DMA patterns.
  - Device, stage 1 (edges): per 128-edge tile,
        mlp = rbf_tile @ W_rbf           (PE, K=16)
        x   = m_tile * mlp               (ACT copy + DVE mult, bf16)
        oh[e,a] = (id[e] == a)           (GPSIMD tensor_scalar vs iota)
        x2_block += oh.T @ x             (PE, accumulated in PSUM)
    Segment-sum needs no collective because the atom range owns all its
    edges (host sorted them).
  - Device, stage 2 (atoms): transpose x2 blocks to feature-major and run
    the 5 dense layers (W1 + 2 residual blocks) weight-stationary across
    blocks, silu on ACT, residual merges on DVE.
  - Host: concat per-core outputs.
"""

import sys

if "/opt/trn_rl_repo" not in sys.path:
    sys.path.insert(0, "/opt/trn_rl_repo")

import numpy as np
import ml_dtypes

N_CORES = 8
N_ATOMS = 50000
N_EDGES = 1600000
D = 256          # edge/atom embedding
R = 16           # rbf dim
P = 128
ATOMS_PER_CORE = N_ATOMS // N_CORES      # 6250
NBLK = (ATOMS_PER_CORE + P - 1) // P     # 49 blocks of 128 atoms
ATOMS_PAD = NBLK * P                     # 6272
INV_SQRT2 = 0.7071067811865476
PAD_ID = 200.0                           # local id that matches no iota value

BF16 = ml_dtypes.bfloat16

_COMPILED = {}   # T -> nc


def _pack_inputs(m, rbf, id_j):
    """Sort edges by atom, shard by atom range, pad per 128-atom block."""
    id_j = np.ascontiguousarray(np.asarray(id_j).astype(np.int64).ravel())
    order = np.argsort(id_j)
    ids_sorted = id_j[order]

    # block atom boundaries for every (core, block)
    blk_bases = []           # absolute atom id at start of each block
    for c in range(N_CORES):
        for b in range(NBLK):
            blk_bases.append(c * ATOMS_PER_CORE + min(b * P, ATOMS_PER_CORE))
    blk_bases.append(N_ATOMS)
    blk_bases = np.asarray(blk_bases, dtype=np.int64)
    bounds = np.searchsorted(ids_sorted, blk_bases)
    cnts = np.diff(bounds).reshape(N_CORES, NBLK)    # edges per (core, block)

    T = int(np.ceil(cnts.max() / P))
    E_blk = T * P
    NT = NBLK * T

    m_bf = np.asarray(m).astype(BF16)
    rbf_bf = np.asarray(rbf).astype(BF16)

    in_parts = []
    for c in range(N_CORES):
        s, e = bounds[c * NBLK], bounds[(c + 1) * NBLK]
        n_c = e - s
        ord_c = order[s:e]
        cnt_c = cnts[c]
        blk_of = np.repeat(np.arange(NBLK), cnt_c)
        # destination slot inside the padded layout
        starts_rel = np.concatenate([[0], np.cumsum(cnt_c)[:-1]])
        j = np.arange(n_c)
        dest = blk_of * E_blk + (j - starts_rel[blk_of])

        m_core = np.zeros((NBLK * E_blk, D), dtype=BF16)
        m_core[dest] = m_bf[ord_c]
        # tile-major: [NBLK, T, P, D] -> [NBLK, P, T, D] -> [NBLK*P, T*D]
        m_core = np.ascontiguousarray(
            m_core.reshape(NBLK, T, P, D).transpose(0, 2, 1, 3)
        ).reshape(NBLK * P, T * D)

        rbf_core = np.zeros((NBLK * E_blk, R), dtype=BF16)
        rbf_core[dest] = rbf_bf[ord_c]
        # [NBLK, T, P, R] -> [NBLK, R, T, P] -> [NBLK*R, T*P]
        rbf_core = np.ascontiguousarray(
            rbf_core.reshape(NBLK, T, P, R).transpose(0, 3, 1, 2)
        ).reshape(NBLK * R, T * P)

        ids_core = np.full(NBLK * E_blk, PAD_ID, dtype=np.float32)
        ids_core[dest] = (ids_sorted[s:e] - (c * ATOMS_PER_CORE + P * blk_of)).astype(
            np.float32
        )
        # [NBLK, T, P] -> [P, NBLK, T] -> [P, NT]
        ids_core = np.ascontiguousarray(
            ids_core.reshape(NBLK, T, P).transpose(2, 0, 1)
        ).reshape(P, NT)

        in_parts.append({"m": m_core, "rbf": rbf_core, "ids": ids_core})
    return in_parts, T


def _pack_weights(W_rbf, W1, W_res):
    w_rbf = np.asarray(W_rbf).astype(BF16)                      # [16, 256]
    layers = [np.asarray(W1)] + [
        np.asarray(W_res[i, l]) for i in range(W_res.shape[0]) for l in range(2)
    ]
    # chunks[l, dc, uc] = W_l[dc*128:, uc*128:]  -> lhsT [K=128 d, M=128 u]
    w2 = np.zeros((5, 2, 2, P, P), dtype=BF16)
    for l, W in enumerate(layers):
        for dc in range(2):
            for uc in range(2):
                w2[l, dc, uc] = W[dc * P : (dc + 1) * P, uc * P : (uc + 1) * P].astype(
                    BF16
                )
    w2 = w2.reshape(20 * P, P)

    iota = np.tile(np.arange(P, dtype=np.float32), (P, 1)).astype(BF16)  # [P, P]
    ident = np.eye(P, dtype=np.float32).astype(BF16)
    return {"w_rbf": w_rbf, "w2": w2, "iota": iota, "ident": ident}


def _build(T):
    from concourse import mybir, bacc
    import concourse.tile as tile

    f32 = mybir.dt.float32
    bf16 = mybir.dt.bfloat16
    AF = mybir.ActivationFunctionType
    ALU = mybir.AluOpType
    NT = NBLK * T

    nc = bacc.Bacc("TRN2", target_bir_lowering=False, debug=False,
                   num_devices=N_CORES)

    m_d = nc.dram_tensor("m", [NBLK * P, T * D], bf16, kind="ExternalInput").ap()
    rbf_d = nc.dram_tensor("rbf", [NBLK * R, T * P], bf16, kind="ExternalInput").ap()
    ids_d = nc.dram_tensor("ids", [P, NT], f32, kind="ExternalInput").ap()
    wrbf_d = nc.dram_tensor("w_rbf", [R, D], bf16, kind="ExternalInput").ap()
    w2_d = nc.dram_tensor("w2", [20 * P, P], bf16, kind="ExternalInput").ap()
    iota_d = nc.dram_tensor("iota", [P, P], bf16, kind="ExternalInput").ap()
    ident_d = nc.dram_tensor("ident", [P, P], bf16, kind="ExternalInput").ap()
    out_d = nc.dram_tensor("out", [2 * P, ATOMS_PAD], f32, kind="ExternalOutput").ap()

    with tile.TileContext(nc) as tc, \
         tc.tile_pool(name="const", bufs=1) as cpool, \
         tc.tile_pool(name="mblk", bufs=2) as mpool, \
         tc.tile_pool(name="rbfblk", bufs=2) as rpool, \
         tc.tile_pool(name="small", bufs=4) as spool, \
         tc.tile_pool(name="ohp", bufs=8) as ohpool, \
         tc.tile_pool(name="big", bufs=1) as bigpool, \
         tc.tile_pool(name="outp", bufs=4) as opool:
        # ---- constants ----
        w_rbf_sb = cpool.tile([R, D], bf16, tag="w_rbf")
        nc.sync.dma_start(w_rbf_sb[:], wrbf_d[:])
        w2_sb = cpool.tile([P, 20 * P], bf16, tag="w2")
        for i in range(20):
            nc.sync.dma_start(
                w2_sb[:, i * P : (i + 1) * P], w2_d[i * P : (i + 1) * P, :]
            )
        iota_sb = cpool.tile([P, P], bf16, tag="iota")
        nc.sync.dma_start(iota_sb[:], iota_d[:])
        ident_sb = cpool.tile([P, P], bf16, tag="ident")
        nc.sync.dma_start(ident_sb[:], ident_d[:])
        ids_sb = cpool.tile([P, NT], f32, tag="ids")
        nc.sync.dma_start(ids_sb[:], ids_d[:])

        def w2c(l, dc, uc):
            i = (l * 2 + dc) * 2 + uc
            return w2_sb[:, i * P : (i + 1) * P]

        # persistent feature-major activations [2 chunks][P, ATOMS_PAD]
        A = [bigpool.tile([P, ATOMS_PAD], bf16, tag=f"A{c}", name=f"A{c}")
             for c in range(2)]
        B = [bigpool.tile([P, ATOMS_PAD], bf16, tag=f"B{c}", name=f"B{c}")
             for c in range(2)]
        C = [bigpool.tile([P, ATOMS_PAD], bf16, tag=f"C{c}", name=f"C{c}")
             for c in range(2)]

        # ---------------- stage 1: edges -> x2^T (into A) ----------------
        with tc.tile_pool(name="ps_mlp", bufs=2, space="PSUM") as ps_mlp, \
             tc.tile_pool(name="ps_x2", bufs=2, space="PSUM") as ps_x2, \
             tc.tile_pool(name="ps_t", bufs=2, space="PSUM") as ps_t:
            for b in range(NBLK):
                m_blk = mpool.tile([P, T * D], bf16, tag="m")
                nc.sync.dma_start(m_blk[:], m_d[b * P : (b + 1) * P, :])
                rbf_blk = rpool.tile([R, T * P], bf16, tag="rbf")
                nc.sync.dma_start(rbf_blk[:], rbf_d[b * R : (b + 1) * R, :])

                x2_ps = ps_x2.tile([P, D], f32, tag="x2")
                V = 4   # edge tiles per macro-group
                for t0 in range(0, T, V):
                    g = min(V, T - t0)
                    mlp_ps = ps_mlp.tile([P, V * D], f32, tag="mlp")
                    for i in range(g):
                        nc.tensor.matmul(
                            mlp_ps[:, i * D : (i + 1) * D],
                            lhsT=rbf_blk[:, (t0 + i) * P : (t0 + i + 1) * P],
                            rhs=w_rbf_sb[:],
                            start=True,
                            stop=True,
                        )
                    mlp_sb = spool.tile([P, V * D], bf16, tag="mlp_sb")
                    nc.scalar.copy(mlp_sb[:, : g * D], mlp_ps[:, : g * D])
                    x_sb = spool.tile([P, V * D], bf16, tag="x_sb")
                    nc.vector.tensor_mul(
                        x_sb[:, : g * D],
                        m_blk[:, t0 * D : (t0 + g) * D],
                        mlp_sb[:, : g * D],
                    )
                    for i in range(g):
                        t = t0 + i
                        oh = ohpool.tile([P, P], bf16, tag="oh")
                        nc.vector.tensor_scalar(
                            oh[:], iota_sb[:], ids_sb[:, b * T + t : b * T + t + 1],
                            None, op0=ALU.is_equal,
                        )
                        nc.tensor.matmul(
                            x2_ps[:], lhsT=oh[:], rhs=x_sb[:, i * D : (i + 1) * D],
                            start=(t == 0), stop=(t == T - 1),
                        )
                x2_sb = spool.tile([P, D], bf16, tag="x2_sb")
                nc.scalar.copy(x2_sb[:], x2_ps[:])
                for ch in range(2):
                    tp = ps_t.tile([P, P], bf16, tag="tp")
                    nc.tensor.transpose(
                        tp[:], x2_sb[:, ch * P : (ch + 1) * P], ident_sb[:]
                    )
                    nc.vector.tensor_copy(A[ch][:, b * P : (b + 1) * P], tp[:])

        # ---------------- stage 2: atom MLP (feature-major) --------------
        with tc.tile_pool(name="ps_s2", bufs=4, space="PSUM") as ps_s2:
            G = 4

            def dense(l, IN, OUT):
                for uc in range(2):
                    for g0 in range(0, NBLK, G):
                        g1 = min(g0 + G, NBLK)
                        pss = [
                            ps_s2.tile([P, P], f32, tag="s2", name=f"s2_{l}_{uc}_{b}")
                            for b in range(g0, g1)
                        ]
                        for dc in range(2):
                            for i, b in enumerate(range(g0, g1)):
                                nc.tensor.matmul(
                                    pss[i][:],
                                    lhsT=w2c(l, dc, uc),
                                    rhs=IN[dc][:, b * P : (b + 1) * P],
                                    start=(dc == 0),
                                    stop=(dc == 1),
                                )
                        for i, b in enumerate(range(g0, g1)):
                            nc.scalar.activation(
                                OUT[uc][:, b * P : (b + 1) * P], pss[i][:], AF.Silu
                            )

            dense(0, A, B)     # h0 = silu(x2 @ W1)
            dense(1, B, C)     # u1 = silu(h0 @ Wa0)
            dense(2, C, A)     # v1 = silu(u1 @ Wb0)
            # merge1: C = (B + A) * inv_sqrt2
            for ch in range(2):
                nc.vector.tensor_add(C[ch][:], B[ch][:], A[ch][:])
                nc.vector.tensor_scalar_mul(C[ch][:], C[ch][:], INV_SQRT2)
            dense(3, C, B)     # u2 = silu(h1 @ Wa1)
            dense(4, B, A)     # v2 = silu(u2 @ Wb1)
            # final: out = (C + A) * inv_sqrt2   (f32), DMA per block
            for ch in range(2):
                for b in range(NBLK):
                    o = opool.tile([P, P], f32, tag="o")
                    nc.vector.tensor_add(
                        o[:], C[ch][:, b * P : (b + 1) * P],
                        A[ch][:, b * P : (b + 1) * P],
                    )
                    nc.vector.tensor_scalar_mul(o[:], o[:], INV_SQRT2)
                    nc.sync.dma_start(
                        out_d[ch * P : (ch + 1) * P, b * P : (b + 1) * P], o[:]
                    )

    nc.compile()
    return nc


def _run(inputs, trace=False):
    from concourse.bass_utils import run_bass_kernel_spmd

    m, rbf, id_j = inputs["m"], inputs["rbf"], inputs["id_j"]
    in_parts, T = _pack_inputs(m, rbf, id_j)
    consts = _pack_weights(inputs["W_rbf"], inputs["W1"], inputs["W_res"])

    if T not in _COMPILED:
        _COMPILED[T] = _build(T)
    nc = _COMPILED[T]

    in_maps = [{**part, **consts} for part in in_parts]
    res = run_bass_kernel_spmd(
        nc, in_maps, core_ids=list(range(N_CORES)), trace=trace
    )
    outs = []
    for c in range(N_CORES):
        o = res.results[c]["out"]                      # [256, ATOMS_PAD]
        outs.append(o[:, :ATOMS_PER_CORE].T)           # [6250, 256]
    full = np.concatenate(outs, axis=0).astype(np.float32)
    return full, res.exec_time_ns


def kernel(**inputs):
    out, _ = _run(inputs, trace=False)
    return out


# revision 17
# speedup vs baseline: 3.8640x; 1.2786x over previous
"""AtomUpdateBlock Trainium2 kernel (8 NeuronCores, SPMD, no collectives).

Strategy:
  - Host: sort edges by destination atom id_j, shard by atom range (6250
    atoms/core).  Within a core, atoms are grouped into 49 blocks of 128;
    each block's edge list is padded to T*128 edges (T = global max tiles
    per block) and packed in tile-major layout for perfect DMA patterns.
    The scatter one-hot matrices are also precomputed host-side (bf16).
  - Device, stage 1 (edges): per 128-edge tile,
        mlp = rbf_tile @ W_rbf      (PE, K=16, 4 tiles packed in row-groups)
        x   = m_tile * mlp          (ACT copy + DVE mult, split by parity)
        x2_block += oh.T @ x        (PE, accumulated in PSUM)
    Segment-sum needs no collective because the atom range owns all its
    edges (host sorted them).
  - Device, stage 2 (atoms): transpose x2 blocks to feature-major and run
    the 5 dense layers (W1 + 2 residual blocks) weight-stationary across
    blocks, silu on ACT, residual merges on DVE.
  - Host: concat per-core outputs.
"""

import sys

if "/opt/trn_rl_repo" not in sys.path:
    sys.path.insert(0, "/opt/trn_rl_repo")

import numpy as np
import ml_dtypes

N_CORES = 8
N_ATOMS = 50000
N_EDGES = 1600000
D = 256          # edge/atom embedding
R = 16           # rbf dim
P = 128
ATOMS_PER_CORE = N_ATOMS // N_CORES      # 6250
NBLK = (ATOMS_PER_CORE + P - 1) // P     # 49 blocks of 128 atoms
ATOMS_PAD = NBLK * P                     # 6272
INV_SQRT2 = 0.7071067811865476

BF16 = ml_dtypes.bfloat16

_COMPILED = {}   # T -> nc


def _pack_inputs(m, rbf, id_j):
    """Sort edges by atom, shard by atom range, pad per 128-atom block."""
    id_j = np.ascontiguousarray(np.asarray(id_j).astype(np.int64).ravel())
    order = np.argsort(id_j)
    ids_sorted = id_j[order]

    blk_bases = []           # absolute atom id at start of each block
    for c in range(N_CORES):
        for b in range(NBLK):
            blk_bases.append(c * ATOMS_PER_CORE + min(b * P, ATOMS_PER_CORE))
    blk_bases.append(N_ATOMS)
    blk_bases = np.asarray(blk_bases, dtype=np.int64)
    bounds = np.searchsorted(ids_sorted, blk_bases)
    cnts = np.diff(bounds).reshape(N_CORES, NBLK)    # edges per (core, block)

    T = int(np.ceil(cnts.max() / P))
    TC = 2 * ((T + 7) // 8)                          # rbf col-blocks (pair-packed)
    E_blk = T * P

    m_bf = np.asarray(m).astype(BF16)
    rbf_bf = np.asarray(rbf).astype(BF16)

    in_parts = []
    for c in range(N_CORES):
        s, e = bounds[c * NBLK], bounds[(c + 1) * NBLK]
        n_c = e - s
        ord_c = order[s:e]
        cnt_c = cnts[c]
        blk_of = np.repeat(np.arange(NBLK), cnt_c)
        starts_rel = np.concatenate([[0], np.cumsum(cnt_c)[:-1]])
        j = np.arange(n_c)
        r = j - starts_rel[blk_of]                   # rank within block
        dest = blk_of * E_blk + r

        m_core = np.zeros((NBLK * E_blk, D), dtype=BF16)
        m_core[dest] = m_bf[ord_c]
        # tile-major: [NBLK, T, P, D] -> [NBLK, P, T, D] -> [NBLK*P, T*D]
        m_core = np.ascontiguousarray(
            m_core.reshape(NBLK, T, P, D).transpose(0, 2, 1, 3)
        ).reshape(NBLK * P, T * D)

        # rbf packed for 4-way row-group matmuls: tile t -> row-group
        # q=(t//2)%4 (rows 32q..32q+16), col block (t//8)*2 + t%2
        rbf_core = np.zeros((NBLK * E_blk, R), dtype=BF16)
        rbf_core[dest] = rbf_bf[ord_c]
        rbf_core = rbf_core.reshape(NBLK, T, P, R)
        rbf4 = np.zeros((NBLK, P, TC * P), dtype=BF16)
        for t in range(T):
            q = (t // 2) % 4
            col = (t // 8) * 2 + (t % 2)
            rbf4[:, 32 * q : 32 * q + R, col * P : (col + 1) * P] = rbf_core[
                :, t
            ].transpose(0, 2, 1)
        rbf4 = rbf4.reshape(NBLK * P, TC * P)

        # one-hot scatter matrices: block slab rows [b*P+e], col t*P + a
        t_of = r // P
        e_of = r % P
        a_of = ids_sorted[s:e] - (c * ATOMS_PER_CORE + P * blk_of)
        oh_core = np.zeros(NBLK * P * T * P, dtype=BF16)
        pos = ((blk_of * P + e_of) * (T * P) + t_of * P + a_of).astype(np.int64)
        oh_core[pos] = 1
        oh_core = oh_core.reshape(NBLK * P, T * P)

        in_parts.append({"m": m_core, "rbf4": rbf4, "oh": oh_core})
    return in_parts, T


def _pack_weights(W_rbf, W1, W_res):
    # W_rbf replicated into the 4 row-group slots (rows 32q..32q+16)
    w_rbf4 = np.zeros((P, D), dtype=BF16)
    for q in range(4):
        w_rbf4[32 * q : 32 * q + R] = np.asarray(W_rbf).astype(BF16)

    layers = [np.asarray(W1)] + [
        np.asarray(W_res[i, l]) for i in range(W_res.shape[0]) for l in range(2)
    ]
    w2 = np.zeros((5, 2, 2, P, P), dtype=BF16)
    for l, W in enumerate(layers):
        for dc in range(2):
            for uc in range(2):
                w2[l, dc, uc] = W[dc * P : (dc + 1) * P, uc * P : (uc + 1) * P].astype(
                    BF16
                )
    w2 = w2.reshape(20 * P, P)

    ident = np.eye(P, dtype=np.float32).astype(BF16)
    return {"w_rbf4": w_rbf4, "w2": w2, "ident": ident}


def _build(T):
    from concourse import mybir, bacc
    import concourse.tile as tile

    f32 = mybir.dt.float32
    bf16 = mybir.dt.bfloat16
    AF = mybir.ActivationFunctionType
    TC = 2 * ((T + 7) // 8)

    nc = bacc.Bacc("TRN2", target_bir_lowering=False, debug=False,
                   num_devices=N_CORES)

    m_d = nc.dram_tensor("m", [NBLK * P, T * D], bf16, kind="ExternalInput").ap()
    rbf_d = nc.dram_tensor("rbf4", [NBLK * P, TC * P], bf16, kind="ExternalInput").ap()
    oh_d = nc.dram_tensor("oh", [NBLK * P, T * P], bf16, kind="ExternalInput").ap()
    wrbf_d = nc.dram_tensor("w_rbf4", [P, D], bf16, kind="ExternalInput").ap()
    w2_d = nc.dram_tensor("w2", [20 * P, P], bf16, kind="ExternalInput").ap()
    ident_d = nc.dram_tensor("ident", [P, P], bf16, kind="ExternalInput").ap()
    out_d = nc.dram_tensor("out", [2 * P, ATOMS_PAD], f32, kind="ExternalOutput").ap()

    with tile.TileContext(nc) as tc, \
         tc.tile_pool(name="const", bufs=1) as cpool, \
         tc.tile_pool(name="mblk", bufs=2) as mpool, \
         tc.tile_pool(name="rbfblk", bufs=2) as rpool, \
         tc.tile_pool(name="ohblk", bufs=2) as ohpool, \
         tc.tile_pool(name="small", bufs=4) as spool, \
         tc.tile_pool(name="big", bufs=1) as bigpool, \
         tc.tile_pool(name="outp", bufs=4) as opool:
        # ---- constants ----
        w_rbf_sb = cpool.tile([P, D], bf16, tag="w_rbf")
        nc.sync.dma_start(w_rbf_sb[:], wrbf_d[:])
        w2_sb = cpool.tile([P, 20 * P], bf16, tag="w2")
        for i in range(20):
            nc.sync.dma_start(
                w2_sb[:, i * P : (i + 1) * P], w2_d[i * P : (i + 1) * P, :]
            )
        ident_sb = cpool.tile([P, P], bf16, tag="ident")
        nc.sync.dma_start(ident_sb[:], ident_d[:])

        def w2c(l, dc, uc):
            i = (l * 2 + dc) * 2 + uc
            return w2_sb[:, i * P : (i + 1) * P]

        # persistent feature-major activations [2 chunks][P, ATOMS_PAD]
        A = [bigpool.tile([P, ATOMS_PAD], bf16, tag=f"A{c}", name=f"A{c}")
             for c in range(2)]
        B = [bigpool.tile([P, ATOMS_PAD], bf16, tag=f"B{c}", name=f"B{c}")
             for c in range(2)]
        C = [bigpool.tile([P, ATOMS_PAD], bf16, tag=f"C{c}", name=f"C{c}")
             for c in range(2)]

        # ---------------- stage 1: edges -> x2^T (into A) ----------------
        with tc.tile_pool(name="ps_mlp", bufs=1, space="PSUM") as ps_mlp, \
             tc.tile_pool(name="ps_x2", bufs=2, space="PSUM") as ps_x2, \
             tc.tile_pool(name="ps_t", bufs=2, space="PSUM") as ps_t:
            V = 8   # edge tiles per macro-group (pairs in 4 row-groups)
            gidx = 0
            for b in range(NBLK):
                m_blk = mpool.tile([P, T * D], bf16, tag="m")
                nc.sync.dma_start(m_blk[:], m_d[b * P : (b + 1) * P, :])
                rbf_blk = rpool.tile([P, TC * P], bf16, tag="rbf")
                nc.sync.dma_start(rbf_blk[:], rbf_d[b * P : (b + 1) * P, :])
                oh_blk = ohpool.tile([P, T * P], bf16, tag="oh")
                nc.sync.dma_start(oh_blk[:], oh_d[b * P : (b + 1) * P, :])

                x2_ps = ps_x2.tile([P, D], f32, tag="x2")
                for t0 in range(0, T, V):
                    g = min(V, T - t0)
                    # one PSUM bank per row-group; bank q holds tiles
                    # t0+2q (j=0) and t0+2q+1 (j=1), serial within a bank
                    psq = [
                        ps_mlp.tile([P, 2 * D], f32, tag=f"mlp{q}",
                                    name=f"mlp_{b}_{t0}_{q}")
                        for q in range(4)
                    ]
                    for i in range(g):
                        t = t0 + i
                        q = (i // 2) % 4
                        j = i % 2
                        col = (t // 8) * 2 + j
                        nc.tensor.matmul(
                            psq[q][:, j * D : (j + 1) * D],
                            lhsT=rbf_blk[32 * q : 32 * q + R, col * P : (col + 1) * P],
                            rhs=w_rbf_sb[32 * q : 32 * q + R, :],
                            start=True,
                            stop=True,
                            tile_position=(32 * q, 0),
                        )
                    x_sb = spool.tile([P, V * D], bf16, tag="x_sb")
                    if gidx % 3 == 0:
                        # fused: DVE mult straight from PSUM (1x rate)
                        for q in range((g + 1) // 2):
                            jn = min(2, g - 2 * q)
                            nc.vector.tensor_mul(
                                x_sb[:, 2 * q * D : (2 * q + jn) * D],
                                m_blk[:, (t0 + 2 * q) * D : (t0 + 2 * q + jn) * D],
                                psq[q][:, : jn * D],
                            )
                    else:
                        # split: ACT copies PSUM->SBUF (per bank), one DVE mult
                        mlp_sb = spool.tile([P, V * D], bf16, tag="mlp_sb")
                        for q in range((g + 1) // 2):
                            jn = min(2, g - 2 * q)
                            nc.scalar.copy(
                                mlp_sb[:, 2 * q * D : (2 * q + jn) * D],
                                psq[q][:, : jn * D],
                            )
                        nc.vector.tensor_mul(
                            x_sb[:, : g * D],
                            m_blk[:, t0 * D : (t0 + g) * D],
                            mlp_sb[:, : g * D],
                        )
                    gidx += 1
                    for i in range(g):
                        t = t0 + i
                        nc.tensor.matmul(
                            x2_ps[:],
                            lhsT=oh_blk[:, t * P : (t + 1) * P],
                            rhs=x_sb[:, i * D : (i + 1) * D],
                            start=(t == 0),
                            stop=(t == T - 1),
                        )
                x2_sb = spool.tile([P, D], bf16, tag="x2_sb")
                nc.scalar.copy(x2_sb[:], x2_ps[:])
                for ch in range(2):
                    tp = ps_t.tile([P, P], bf16, tag="tp")
                    nc.tensor.transpose(
                        tp[:], x2_sb[:, ch * P : (ch + 1) * P], ident_sb[:]
                    )
                    nc.vector.tensor_copy(A[ch][:, b * P : (b + 1) * P], tp[:])

        # ---------------- stage 2: atom MLP (feature-major) --------------
        with tc.tile_pool(name="ps_s2", bufs=4, space="PSUM") as ps_s2:
            G = 4

            def dense(l, IN, OUT):
                for uc in range(2):
                    for g0 in range(0, NBLK, G):
                        g1 = min(g0 + G, NBLK)
                        pss = [
                            ps_s2.tile([P, P], f32, tag="s2", name=f"s2_{l}_{uc}_{b}")
                            for b in range(g0, g1)
                        ]
                        for dc in range(2):
                            for i, b in enumerate(range(g0, g1)):
                                nc.tensor.matmul(
                                    pss[i][:],
                                    lhsT=w2c(l, dc, uc),
                                    rhs=IN[dc][:, b * P : (b + 1) * P],
                                    start=(dc == 0),
                                    stop=(dc == 1),
                                )
                        for i, b in enumerate(range(g0, g1)):
                            nc.scalar.activation(
                                OUT[uc][:, b * P : (b + 1) * P], pss[i][:], AF.Silu
                            )

            dense(0, A, B)     # h0 = silu(x2 @ W1)
            dense(1, B, C)     # u1 = silu(h0 @ Wa0)
            dense(2, C, A)     # v1 = silu(u1 @ Wb0)
            # merge1: C = (B + A) * inv_sqrt2
            for ch in range(2):
                nc.vector.tensor_add(C[ch][:], B[ch][:], A[ch][:])
                nc.vector.tensor_scalar_mul(C[ch][:], C[ch][:], INV_SQRT2)
            dense(3, C, B)     # u2 = silu(h1 @ Wa1)
            dense(4, B, A)     # v2 = silu(u2 @ Wb1)
            # final: out = (C + A) * inv_sqrt2   (f32), DMA per block
            for ch in range(2):
                for b in range(NBLK):
                    o = opool.tile([P, P], f32, tag="o")
                    nc.vector.tensor_add(
                        o[:], C[ch][:, b * P : (b + 1) * P],
                        A[ch][:, b * P : (b + 1) * P],
                    )
                    nc.vector.tensor_scalar_mul(o[:], o[:], INV_SQRT2)
                    nc.sync.dma_start(
                        out_d[ch * P : (ch + 1) * P, b * P : (b + 1) * P], o[:]
                    )

    nc.compile()
    return nc


def _run(inputs, trace=False):
    from concourse.bass_utils import run_bass_kernel_spmd

    m, rbf, id_j = inputs["m"], inputs["rbf"], inputs["id_j"]
    in_parts, T = _pack_inputs(m, rbf, id_j)
    consts = _pack_weights(inputs["W_rbf"], inputs["W1"], inputs["W_res"])

    if T not in _COMPILED:
        _COMPILED[T] = _build(T)
    nc = _COMPILED[T]

    in_maps = [{**part, **consts} for part in in_parts]
    res = run_bass_kernel_spmd(
        nc, in_maps, core_ids=list(range(N_CORES)), trace=trace
    )
    outs = []
    for c in range(N_CORES):
        o = res.results[c]["out"]                      # [256, ATOMS_PAD]
        outs.append(o[:, :ATOMS_PER_CORE].T)           # [6250, 256]
    full = np.concatenate(outs, axis=0).astype(np.float32)
    return full, res.exec_time_ns


def kernel(**inputs):
    out, _ = _run(inputs, trace=False)
    return out
